# revision 1
# baseline (speedup 1.0000x reference)
"""CCSDS123 lossless compressor kernel for Trainium2 (8 NeuronCores).

Strategy
--------
The reference is, per band z, a strictly sequential sign-LMS scan over the
Y*X = 65536 raster positions with a 3-vector adaptive weight state; bands
are independent (band z only reads raw samples of bands z-1..z-3).

Sharding: bands are padded 202 -> 208 = 8*26 and each core owns 26
contiguous bands plus a 3-band halo of raw samples.

On-device, each core runs its 26 band-scans in lockstep (bands on
partitions); each sequential step is exactly TWO vector-engine
instructions:

  1) LMS_DOTSIGN_ANT (custom DVE op): streams (mu*n_t, w'_t), forms the
     running prefix dot via an in-instruction scan, clips against
     [-32768, 32767], compares against the sample s_t, and emits
     sigma = sign(s_t - clip(pred)) as the last streamed element.
  2) stock scalar_tensor_tensor: w'_{t+1} = (n_t * sigma) + w'_t, written
     to a w-trajectory buffer.

Weights are tracked scaled (w' = w / MU).  Because MU = 2^-10 is a power
of two, every f32 operation commutes exactly with this scaling, so the
computation is bit-identical to the reference recurrence.

Predictions/residuals are recomputed OFF the critical engine (bulk ops on
the Pool engine) from the w-trajectory: a_t = sum_p w'_{t,p} * (MU n_{t,p}),
pred = clip(a), res = s - pred, DMA'd out per chunk.

The remaining reference outputs are trivial functions computed on host:
quantized_residuals = residuals, sample_representatives = image,
reconstructed = pred + res, mapped indices from rint(res).

Sync-wait budget: walrus can encode only a couple of semaphore waits per
instruction, so the chunk dataflow is arranged so no instruction depends
on more than 2 cross-engine producers: all neighbor planes + samples
arrive in ONE overlapping strided DMA ([26,4,C]: pages = rows z,z+1,z+2,
z+3 of the halo'd band array), mu-scaled planes in a second DMA from a
host-precomputed mu*bands array, and pred/res leave in one DMA.
"""

import numpy as np

MU = 2.0 ** -10
P = 3
ZB = 26            # bands per core
NCORES = 8
ZPAD = ZB * NCORES  # 208
Y = 256
X = 256
T_FULL = Y * X     # 65536

_OP_NAME = "LMS_DOTSIGN_ANT"
_build_cache = {}


def _register_dve_op():
    """Register the fused dot+clip+sign custom DVE op (idempotent)."""
    import concourse.dve_ops as dve_ops
    from concourse.dve_spec import Spec, Src0, Src1, C0, C1, C2, AluOp, scan, maxx, minn, lower
    from concourse.dve_uop import DveOpSpec

    for op in dve_ops.OPS:
        if op.name == _OP_NAME:
            return op

    # out[p,k] = sign(s0 - clip(prefix_k(Src0*Src1), s1, imm2))
    #   Src0 = mu*n_t (3 elems), Src1 = w'_t (3 elems),
    #   s0 = sample s_t [P,1], s1 = lo = -32768.0 [P,1] tile, imm2 = 32767.0
    # The element at k=2 is the true sigma_t.
    pr = scan(AluOp.ADD, Src0 * Src1)
    pclip = minn(maxx(pr, C1), C2)
    body = (C0 > pclip) - (C0 < pclip)

    def _ref(in0, in1, s0, s1, imm2):
        prods = (np.asarray(in0, np.float32) * np.asarray(in1, np.float32)).astype(np.float32)
        pr = np.cumsum(prods, axis=-1, dtype=np.float32)
        pp = np.minimum(np.maximum(pr, np.float32(s1)), np.float32(imm2)).astype(np.float32)
        s0 = np.asarray(s0, np.float32)
        return (np.greater(s0, pp).astype(np.float32)
                - np.less(s0, pp).astype(np.float32))

    spec = Spec(body=body, reference=_ref)
    row = dve_ops._CUSTOM_DVE_ROW_BASE + len(dve_ops.OPS)
    shas = {}
    for ver in ("v3", "v4"):
        uops = lower(spec, ver=ver)
        shas[ver] = DveOpSpec(name=_OP_NAME, opcode=row, uops=uops, rd1_en=True).sha(ver)
    op = dve_ops.DveOp(_OP_NAME, spec, subdim=False, uops_sha=shas)
    dve_ops.OPS.append(op)
    dve_ops.CUSTOM_DVE_SPECS[op.name] = spec
    dve_ops._SUB_OPCODE_FOR_NAME[op.name] = row
    return op


def _restride(ap, free_pairs):
    """Return ap with its free-dim [step, count] pairs replaced."""
    import bass_rust
    part = ap.ap.to_list()[0]
    ap.ap = bass_rust.VecI64Pair([part] + [list(p) for p in free_pairs])
    return ap


def _strip_dve_self_sems(nc):
    """Remove the per-instruction DVE self-semaphore chain Tile emits for
    same-engine dependencies inside the big scan blocks.

    The DVE executes its stream in order and ends every op with a pipeline
    DRAIN (the output-dependency barrier), so a DVE instruction never needs
    a semaphore wait on the DVE's own engine semaphore.  Dropping the
    wait+update pair on the scan instructions removes a ~1.4us/instr
    serialization.  Updates that non-DVE consumers (Pool, the loop drain)
    actually wait on are kept, and those consumers' thresholds are rewritten
    to the new cumulative counts.
    """
    import bass_rust

    for f in nc.m.functions:
        blocks = list(f.blocks)
        big_idx = max(range(len(blocks)), key=lambda k: len(blocks[k].instructions))
        big = blocks[big_idx]
        is_dve = lambda i: "DVE" in str(i.engine)
        is_branch = lambda i: "Branch" in str(i.opcode)

        sem_ids = set()
        sem_name = {}
        for b in blocks:
            for i in b.instructions:
                if is_dve(i) and i.sync_info is not None:
                    for u in i.sync_info.on_update:
                        if u.ant_name and u.ant_name.startswith("DVE"):
                            sem_ids.add(u.id)
                            sem_name[u.id] = u.ant_name

        for sem in sem_ids:
            total_body = 0
            for i in big.instructions:
                if i.sync_info is not None:
                    total_body += sum(u.update_value for u in i.sync_info.on_update
                                      if u.id == sem)
            if total_body == 0:
                continue

            # 1. DVE instructions never need to wait on the DVE's own sem
            for b in blocks:
                for i in b.instructions:
                    si = i.sync_info
                    if si is None or not is_dve(i):
                        continue
                    if any(w.id == sem for w in si.on_wait):
                        si.on_wait = [w for w in si.on_wait if w.id != sem]

            # updates stay exactly as Tile emitted them (fire-and-forget;
            # totals and the loop's wraparound top-up remain valid) — only
            # the per-instruction self-waits are removed.


def _build(T, C, pair_loop):
    """Build the SPMD Bass program.

    T: total steps per band; C: chunk size; chunks are processed in
    statically-unrolled pairs inside a dynamic For_i loop when pair_loop,
    else fully statically.
    """
    import concourse.bass as bass
    import concourse.bacc as bacc
    import concourse.mybir as mybir
    from concourse.tile import TileContext

    key = (T, C, pair_loop)
    if key in _build_cache:
        return _build_cache[key]

    op = _register_dve_op()
    nchunks = T // C
    assert T % C == 0 and nchunks % 2 == 0

    nc = bacc.Bacc(trn_type="TRN2", detect_race_conditions=False)
    f32 = mybir.dt.float32
    add = mybir.AluOpType.add
    mult = mybir.AluOpType.mult
    amin = mybir.AluOpType.min
    amax = mybir.AluOpType.max

    bands = nc.dram_tensor("bands", [ZB + P, T], f32, kind="ExternalInput")
    mubands = nc.dram_tensor("mubands", [ZB + P - 1, T], f32, kind="ExternalInput")
    w0s = nc.dram_tensor("w0s", [ZB, P], f32, kind="ExternalInput")
    predres_o = nc.dram_tensor("predres", [2, ZB, T], f32, kind="ExternalOutput")

    with TileContext(nc) as tc:
        # nbs pages: 0 = plane p=2 (band z-3), 1 = plane p=1, 2 = plane p=0,
        #            3 = sample s  (source rows z, z+1, z+2, z+3 — one DMA)
        # mbr pages: 0..2 = mu*plane (reversed p like nbs pages 0..2)
        nbs = nc.alloc_sbuf_tensor("nbs", [ZB, 4, C], f32).ap()
        mbr = nc.alloc_sbuf_tensor("mbr", [ZB, 3, C], f32).ap()
        wtraj = nc.alloc_sbuf_tensor("wtraj", [ZB, C + 1, P], f32).ap()
        predres = nc.alloc_sbuf_tensor("predres_sb", [ZB, 2, C], f32).ap()
        prods = nc.alloc_sbuf_tensor("prods", [ZB, C, P], f32).ap()
        a_t = nc.alloc_sbuf_tensor("a", [ZB, C], f32).ap()
        sg = nc.alloc_sbuf_tensor("sg", [ZB, P], f32).ap()
        lo_t = nc.alloc_sbuf_tensor("lo", [ZB, 1], f32).ap()

        nc.vector.memset(lo_t[:, :], -32768.0)
        # seed: the carry-copy below reads "previous chunk's last w slot"
        nc.sync.dma_start(wtraj[:, C, :], w0s[:, :])

        def do_chunk(col0):
            # one overlapping DMA: [26, 4, C] <- bands rows [z .. z+3]
            src = _restride(bands[0:ZB, bass.ds(col0, C)].unsqueeze(1),
                            [[T, 4], [1, C]])
            nc.sync.dma_start(nbs[:, :, :], src)
            msrc = _restride(mubands[0:ZB, bass.ds(col0, C)].unsqueeze(1),
                             [[T, 3], [1, C]])
            nc.sync.dma_start(mbr[:, :, :], msrc)

            s_col = nbs[:, 3, :]
            # carry w' into slot 0 (on DVE: keeps the scan single-engine)
            nc.vector.tensor_copy(wtraj[:, 0, :], wtraj[:, C, :])

            # ---- the sequential scan: 2 DVE instructions per step ----
            for t in range(C):
                # mu*n_t in p-order: mbr pages 2,1,0 at column t (stride -C)
                mu_n_t = _restride(mbr[:, 2, t:t + 1], [[-C, 3]])
                n_t = _restride(nbs[:, 2, t:t + 1], [[-C, 3]])
                nc.vector._custom_dve(
                    op,
                    out=sg[:, :],
                    in0=mu_n_t,
                    in1=wtraj[:, t, :],
                    s0=s_col[:, t:t + 1],
                    s1=-32768.0,
                    imm2=32767.0,
                )
                nc.vector.scalar_tensor_tensor(
                    out=wtraj[:, t + 1, :],
                    in0=n_t,
                    scalar=sg[:, 2:3],
                    in1=wtraj[:, t, :],
                    op0=mult,
                    op1=add,
                )

            # ---- bulk reconstruction off the DVE (Pool engine) ----
            mbr_cp = _restride(mbr[:, 2, 0:C], [[1, C], [-C, 3]])
            nc.gpsimd.tensor_tensor(prods[:, :, :], mbr_cp,
                                    wtraj[:, 0:C, :], mult)
            nc.gpsimd.tensor_tensor(a_t[:, :], prods[:, :, 0], prods[:, :, 1], add)
            nc.gpsimd.tensor_tensor(a_t[:, :], a_t[:, :], prods[:, :, 2], add)
            nc.gpsimd.tensor_scalar(predres[:, 0, :], a_t[:, :],
                                    32767.0, -32768.0, op0=amin, op1=amax)
            nc.gpsimd.tensor_tensor(predres[:, 1, :], s_col,
                                    predres[:, 0, :], mybir.AluOpType.subtract)
            dst = _restride(predres_o[0, 0:ZB, bass.ds(col0, C)].unsqueeze(1),
                            [[ZB * T, 2], [1, C]])
            nc.sync.dma_start(dst, predres[:, :, :])

        if pair_loop:
            with tc.For_i(0, nchunks, 1) as i:
                do_chunk(i * C)
        else:
            for k in range(nchunks):
                do_chunk(k * C)

    if not nc.is_finalized():
        nc.finalize()
    _build_cache[key] = nc
    return nc


def _run_spmd(nc, in_maps, trace=False):
    from concourse.bass_utils import run_bass_kernel_spmd
    return run_bass_kernel_spmd(nc, in_maps, core_ids=list(range(len(in_maps))),
                                trace=trace)


def _make_in_maps(G, w0p):
    """G: [3+ZPAD, T] halo'd band planes; w0p: [ZPAD, 3] scaled weights."""
    mug = (np.float32(MU) * G).astype(np.float32)
    in_maps = []
    for c in range(NCORES):
        in_maps.append({
            "bands": np.ascontiguousarray(G[ZB * c:ZB * c + ZB + P]),
            "mubands": np.ascontiguousarray(mug[ZB * c:ZB * c + ZB + P - 1]),
            "w0s": np.ascontiguousarray(w0p[ZB * c:ZB * c + ZB]),
        })
    return in_maps


def kernel(image, w0):
    image = np.ascontiguousarray(np.asarray(image, np.float32))
    w0 = np.asarray(w0, np.float32)
    Z = image.shape[0]
    img_flat = image.reshape(Z, T_FULL)

    G = np.zeros((P + ZPAD, T_FULL), np.float32)
    G[P:P + Z] = img_flat
    w0p = np.zeros((ZPAD, P), np.float32)
    w0p[:Z] = w0 * np.float32(1024.0)

    nc = _build(T_FULL, 2048, True)
    results = _run_spmd(nc, _make_in_maps(G, w0p)).results

    preds = np.concatenate([r["predres"][0] for r in results], axis=0)[:Z]
    ress = np.concatenate([r["predres"][1] for r in results], axis=0)[:Z]
    predictions = preds.reshape(Z, Y, X)
    residuals = ress.reshape(Z, Y, X)

    q = np.rint(residuals).astype(np.int32)
    mapped = np.where(q >= 0, 2 * q, -2 * q - 1).astype(np.int32)
    reconstructed = predictions + residuals
    return (predictions, residuals, residuals, mapped, image, reconstructed)



# revision 2
# speedup vs baseline: 9.6632x; 9.6632x over previous
"""CCSDS123 kernel v2: ONE fused DVE instruction per sequential step.

The per-step recurrence (exact f32, scaled weights w' = w/MU):
    pr    = ((w0*mu_n0) + w1*mu_n1) + w2*mu_n2        [exact reference order]
    sigma = (s > pr) - (s < pr)     [== sign(s - clip(pr)) since |s| < 32768]
    w'    = w' + sigma * n                            [exact]

Implemented as a hand-crafted 3-uop DVE FSM (seed, A, B):
  seed (1 cycle, no consume): zero the prefix-sum flop (block b1).
  A (3 elements): b0 prod = Src0*Src1; b1 pr += prod (CURR feedback);
     b2 g1 = pr < s; b3 g2 = s < pr; b4 sigma = g1 - g2. After element 2,
     b4's flop holds the true sigma.
  B (3 elements): b1/b4 BYPASS(CURR) freeze pr and sigma; b5 = sigma*Src0
     (Src0 now carries raw n_p); b6 = b5 + Src1 (w_p); written out.

Streams (6 elements each):
  Src0 = [mu_n0, mu_n1, mu_n2, n0, n1, n2]   (combined buffer, strided AP)
  Src1 = [w0, w1, w2, w0, w1, w2]            (stride-0 outer AP on wtraj)
  out  = wtraj[:, t+1, :] with [[0,2],[1,3]]: A's 3 garbage writes land on
         the same 3 addresses B then overwrites.

Everything else (sharding, Pool reconstruction of pred/res, host pre/post)
matches the baseline kernel.
"""

import numpy as np

MU = 2.0 ** -10
P = 3
ZB = 26
NCORES = 8
ZPAD = ZB * NCORES
Y = 256
X = 256
T_FULL = Y * X

_OP_NAME = "LMS_FUSED_STEP_ANT"
_build_cache = {}


def _make_fused_uops():
    from concourse.dve_uop import (
        UopConfig, UopDpConfig, AluOp, AluInp, InpSel, OutSel, OutPath,
        Trigger, DelayInp, ENABLE,
    )

    PREV = AluInp.PREV_ALU_OUT
    CURR = AluInp.CURR_ALU_OUT
    L0, L1, L2, L3, L4, L5 = (AluInp.PREV_DELAY_0, AluInp.PREV_DELAY_1,
                              AluInp.PREV_DELAY_2, AluInp.PREV_DELAY_3,
                              AluInp.PREV_DELAY_4, AluInp.PREV_DELAY_5)

    def base_uop():
        u = UopConfig()
        u.enable_input(InpSel.SRC_0, 1)     # lane 0
        u.enable_input(InpSel.SRC_1, 2)     # lane 1
        u.enable_input(InpSel.CONST_0, 3)   # lane 2: s_t
        u.enable_input(InpSel.ZERO, 4)      # lane 3: seed zero
        for b in range(8):
            u.datapath_config[b].pass_through_delay(0, 1, 2, 3, 4, 5)
        return u

    def a_blocks(u):
        dp = u.datapath_config
        dp[0].enable_alu(AluOp.MULTIPLY, L0, L1)
        dp[1].enable_alu(AluOp.ADD, CURR, PREV)
        dp[2].enable_alu(AluOp.IS_LT, PREV, L2)       # g1 = pr < s
        dp[2].enable_delay_from_src(DelayInp.PREV_ALU_OUT, 4)   # lane4 <- pr
        dp[3].enable_alu(AluOp.IS_LT, L2, L4)         # g2 = s < pr
        dp[3].enable_delay_from_src(DelayInp.PREV_ALU_OUT, 5)   # lane5 <- g1
        dp[4].enable_alu(AluOp.SUBTRACT, L5, PREV)    # sigma = g1 - g2
        dp[5].enable_alu(AluOp.BYPASS, PREV)
        dp[6].enable_alu(AluOp.BYPASS, PREV)
        dp[7].enable_alu(AluOp.BYPASS, PREV)

    # uop0: seed — zero the pr flop; run the A datapath otherwise; no
    # consume, no write (mirrors lower()'s seed-state conventions).
    seed = base_uop()
    a_blocks(seed)
    seed.datapath_config[1].enable_alu(AluOp.BYPASS, L3, L3)
    seed.trigger = (Trigger.COUNT, Trigger.NONE, Trigger.NONE)
    seed.next_uop = (1, 0, 0)
    seed.repeat_count = 1

    # uop1: A — 3 dot elements, then COUNT -> B
    ua = base_uop()
    a_blocks(ua)
    ua.trigger = (Trigger.SRC_TENSOR_DONE, Trigger.NONE, Trigger.COUNT)
    ua.next_uop = (0, 0, 2)
    ua.repeat_count = 3
    ua.require_inp0 = 1
    ua.require_inp1 = 1
    ua.enable_output(OutSel.ALU_OUT, OutPath.WR0_LO)

    # uop2: B — 3 update elements until the stream ends
    ub = base_uop()
    dp = ub.datapath_config
    dp[0].enable_alu(AluOp.BYPASS, PREV)
    dp[1].enable_alu(AluOp.BYPASS, CURR, CURR)        # freeze pr
    dp[2].enable_alu(AluOp.BYPASS, PREV)
    dp[3].enable_alu(AluOp.BYPASS, PREV)
    dp[4].enable_alu(AluOp.BYPASS, CURR, CURR)        # hold sigma
    dp[5].enable_alu(AluOp.MULTIPLY, PREV, L0)        # sigma * n_p
    dp[6].enable_alu(AluOp.ADD, PREV, L1)             # + w_p
    dp[7].enable_alu(AluOp.BYPASS, PREV)
    ub.trigger = (Trigger.SRC_TENSOR_DONE, Trigger.NONE, Trigger.NONE)
    ub.next_uop = (0, 0, 0)
    ub.require_inp0 = 1
    ub.require_inp1 = 1
    ub.enable_output(OutSel.ALU_OUT, OutPath.WR0_LO)

    uops = [seed, ua, ub]
    for u in uops:
        u.validate("v3")
    return uops


def _register_fused_op():
    """Register the fused op; inject hand-crafted uops into the compile
    cache so table-gen uses them instead of lowering the spec."""
    import concourse.dve_ops as dve_ops
    from concourse.dve_ops import _COMPILE_CACHE
    from concourse.dve_spec import Spec, Src0, Src1, C0, AluOp, scan
    from concourse.dve_uop import DveOpSpec

    for op in dve_ops.OPS:
        if op.name == _OP_NAME:
            return op

    # Spec body documents the A-phase semantic (clipless sigma); it also
    # determines rd1_en (uses Src1) and the scalar slots (C0 only — C2-free
    # so the 2-free-dim in1 STT encoding is allowed). The actual uops are
    # injected below.
    pr = scan(AluOp.ADD, Src0 * Src1)
    body = (C0 > pr) - (C0 < pr)

    def _ref(in0, in1, s0, s1, imm2):
        in0 = np.asarray(in0, np.float32)
        in1 = np.asarray(in1, np.float32)
        pr = np.cumsum((in0 * in1).astype(np.float32), axis=-1,
                       dtype=np.float32)
        s0 = np.asarray(s0, np.float32)
        return (np.greater(s0, pr).astype(np.float32)
                - np.less(s0, pr).astype(np.float32))

    spec = Spec(body=body, reference=_ref)
    row = dve_ops._CUSTOM_DVE_ROW_BASE + len(dve_ops.OPS)
    uops = _make_fused_uops()
    shas = {}
    for ver in ("v3", "v4"):
        compiled = DveOpSpec(name=_OP_NAME, opcode=row, uops=uops, rd1_en=True)
        shas[ver] = compiled.sha(ver)
        _COMPILE_CACHE[(_OP_NAME, ver)] = compiled
    op = dve_ops.DveOp(_OP_NAME, spec, subdim=False, uops_sha=shas)
    dve_ops.OPS.append(op)
    dve_ops.CUSTOM_DVE_SPECS[op.name] = spec
    dve_ops._SUB_OPCODE_FOR_NAME[op.name] = row
    return op


def _restride(ap, free_pairs):
    import bass_rust
    part = ap.ap.to_list()[0]
    ap.ap = bass_rust.VecI64Pair([part] + [list(p) for p in free_pairs])
    return ap


def _strip_dve_self_sems(nc):
    """Drop per-instruction DVE self-semaphore waits (program order + the
    DVE's end-of-op drain already give the ordering); keep all updates."""
    for f in nc.m.functions:
        blocks = list(f.blocks)
        is_dve = lambda i: "DVE" in str(i.engine)
        sem_ids = set()
        for b in blocks:
            for i in b.instructions:
                if is_dve(i) and i.sync_info is not None:
                    for u in i.sync_info.on_update:
                        if u.ant_name and u.ant_name.startswith("DVE"):
                            sem_ids.add(u.id)
        for b in blocks:
            for i in b.instructions:
                si = i.sync_info
                if si is None or not is_dve(i):
                    continue
                if any(w.id in sem_ids for w in si.on_wait):
                    si.on_wait = [w for w in si.on_wait if w.id not in sem_ids]


def _build(T, C, pair_loop, reps=1):
    import concourse.bass as bass
    import concourse.bacc as bacc
    import concourse.mybir as mybir
    from concourse.tile import TileContext

    key = (T, C, pair_loop, reps)
    if key in _build_cache:
        return _build_cache[key]

    op = _register_fused_op()
    nchunks = T // C
    assert T % C == 0 and (nchunks % 2 == 0 or not pair_loop)

    nc = bacc.Bacc(trn_type="TRN2", detect_race_conditions=False)
    f32 = mybir.dt.float32
    add = mybir.AluOpType.add
    mult = mybir.AluOpType.mult
    amin = mybir.AluOpType.min
    amax = mybir.AluOpType.max

    bands = nc.dram_tensor("bands", [ZB + P, T], f32, kind="ExternalInput")
    mubands = nc.dram_tensor("mubands", [ZB + P - 1, T], f32, kind="ExternalInput")
    w0s = nc.dram_tensor("w0s", [ZB, P], f32, kind="ExternalInput")
    predres_o = nc.dram_tensor("predres", [2, ZB, T], f32, kind="ExternalOutput")

    with TileContext(nc) as tc:
        # cmb pages: 0..2 = mu*rows z..z+2, 4..7 = raw rows z..z+3
        #   for band z: mu_n_p = page (2-p), n_p = page (6-p), sample = page 7
        cmb = nc.alloc_sbuf_tensor("cmb", [ZB, 8, C], f32).ap()
        wtraj = nc.alloc_sbuf_tensor("wtraj", [ZB, C + 1, P], f32).ap()
        predres = nc.alloc_sbuf_tensor("predres_sb", [ZB, 2, C], f32).ap()
        prods = nc.alloc_sbuf_tensor("prods", [ZB, C, P], f32).ap()
        a_t = nc.alloc_sbuf_tensor("a", [ZB, C], f32).ap()

        # seed: carry-copy below reads "previous chunk's last w slot"
        nc.sync.dma_start(wtraj[:, C, :], w0s[:, :])

        def do_chunk(col0):
            src = _restride(bands[0:ZB, bass.ds(col0, C)].unsqueeze(1),
                            [[T, 4], [1, C]])
            nc.sync.dma_start(cmb[:, 4:8, :], src)
            msrc = _restride(mubands[0:ZB, bass.ds(col0, C)].unsqueeze(1),
                             [[T, 3], [1, C]])
            nc.sync.dma_start(cmb[:, 0:3, :], msrc)

            s_col = cmb[:, 7, :]
            # carry w' into slot 0 (on DVE: keeps the chain single-engine)
            nc.vector.tensor_copy(wtraj[:, 0, :], wtraj[:, C, :])

            # ---- the sequential scan: ONE fused DVE instruction per step ----
            for t in range(C):
                in0 = _restride(cmb[:, 2, t:t + 1], [[4 * C, 2], [-C, 3]])
                in1 = _restride(wtraj[:, t, :].unsqueeze(1), [[0, 2], [1, 3]])
                outw = _restride(wtraj[:, t + 1, :].unsqueeze(1),
                                 [[0, 2], [1, 3]])
                nc.vector._custom_dve(
                    op,
                    out=outw,
                    in0=in0,
                    in1=in1,
                    s0=s_col[:, t:t + 1],
                    s1=0.0,
                )

            # ---- bulk reconstruction off the DVE (Pool engine) ----
            mbr_cp = _restride(cmb[:, 2, 0:C], [[1, C], [-C, 3]])
            nc.gpsimd.tensor_tensor(prods[:, :, :], mbr_cp,
                                    wtraj[:, 0:C, :], mult)
            nc.gpsimd.tensor_tensor(a_t[:, :], prods[:, :, 0], prods[:, :, 1], add)
            nc.gpsimd.tensor_tensor(a_t[:, :], a_t[:, :], prods[:, :, 2], add)
            nc.gpsimd.tensor_scalar(predres[:, 0, :], a_t[:, :],
                                    32767.0, -32768.0, op0=amin, op1=amax)
            nc.gpsimd.tensor_tensor(predres[:, 1, :], s_col,
                                    predres[:, 0, :], mybir.AluOpType.subtract)
            dst = _restride(predres_o[0, 0:ZB, bass.ds(col0, C)].unsqueeze(1),
                            [[ZB * T, 2], [1, C]])
            nc.sync.dma_start(dst, predres[:, :, :])

        if pair_loop and reps > 1:
            with tc.For_i(0, reps, 1):
                with tc.For_i(0, nchunks, 1) as i:
                    do_chunk(i * C)
        elif pair_loop:
            with tc.For_i(0, nchunks, 1) as i:
                do_chunk(i * C)
        else:
            for k in range(nchunks):
                do_chunk(k * C)

    _strip_dve_self_sems(nc)
    if not nc.is_finalized():
        nc.finalize()
    _build_cache[key] = nc
    return nc


def _run_spmd(nc, in_maps, trace=False):
    from concourse.bass_utils import run_bass_kernel_spmd
    return run_bass_kernel_spmd(nc, in_maps, core_ids=list(range(len(in_maps))),
                                trace=trace)


def _make_in_maps(G, w0p):
    mug = (np.float32(MU) * G).astype(np.float32)
    in_maps = []
    for c in range(NCORES):
        in_maps.append({
            "bands": np.ascontiguousarray(G[ZB * c:ZB * c + ZB + P]),
            "mubands": np.ascontiguousarray(mug[ZB * c:ZB * c + ZB + P - 1]),
            "w0s": np.ascontiguousarray(w0p[ZB * c:ZB * c + ZB]),
        })
    return in_maps


def kernel(image, w0):
    image = np.ascontiguousarray(np.asarray(image, np.float32))
    w0 = np.asarray(w0, np.float32)
    Z = image.shape[0]
    img_flat = image.reshape(Z, T_FULL)
    assert float(np.abs(img_flat).max()) < 32000.0, \
        "clipless sigma requires |s| < 32768"

    G = np.zeros((P + ZPAD, T_FULL), np.float32)
    G[P:P + Z] = img_flat
    w0p = np.zeros((ZPAD, P), np.float32)
    w0p[:Z] = w0 * np.float32(1024.0)

    nc = _build(T_FULL, 2048, True)
    results = _run_spmd(nc, _make_in_maps(G, w0p)).results

    preds = np.concatenate([r["predres"][0] for r in results], axis=0)[:Z]
    ress = np.concatenate([r["predres"][1] for r in results], axis=0)[:Z]
    predictions = preds.reshape(Z, Y, X)
    residuals = ress.reshape(Z, Y, X)

    q = np.rint(residuals).astype(np.int32)
    mapped = np.where(q >= 0, 2 * q, -2 * q - 1).astype(np.int32)
    reconstructed = predictions + residuals
    return (predictions, residuals, residuals, mapped, image, reconstructed)


# revision 3
# speedup vs baseline: 9.8755x; 1.0220x over previous
"""CCSDS123 kernel v5 (cmb double-buffered): ONE fused DVE instruction per sequential step.

The per-step recurrence (exact f32, scaled weights w' = w/MU):
    pr    = ((w0*mu_n0) + w1*mu_n1) + w2*mu_n2        [exact reference order]
    sigma = (s > pr) - (s < pr)     [== sign(s - clip(pr)) since |s| < 32768]
    w'    = w' + sigma * n                            [exact]

Implemented as a hand-crafted 3-uop DVE FSM (seed, A, B):
  seed (1 cycle, no consume): zero the prefix-sum flop (block b1).
  A (3 elements): b0 prod = Src0*Src1; b1 pr += prod (CURR feedback);
     b2 g1 = pr < s; b3 g2 = s < pr; b4 sigma = g1 - g2. After element 2,
     b4's flop holds the true sigma.
  B (3 elements): b1/b4 BYPASS(CURR) freeze pr and sigma; b5 = sigma*Src0
     (Src0 now carries raw n_p); b6 = b5 + Src1 (w_p); written out.

Streams (6 elements each):
  Src0 = [mu_n0, mu_n1, mu_n2, n0, n1, n2]   (combined buffer, strided AP)
  Src1 = [w0, w1, w2, w0, w1, w2]            (stride-0 outer AP on wtraj)
  out  = wtraj[:, t+1, :] with [[0,2],[1,3]]: A's 3 garbage writes land on
         the same 3 addresses B then overwrites.

Everything else (sharding, Pool reconstruction of pred/res, host pre/post)
matches the baseline kernel.
"""

import numpy as np

MU = 2.0 ** -10
P = 3
ZB = 26
NCORES = 8
ZPAD = ZB * NCORES
Y = 256
X = 256
T_FULL = Y * X

_OP_NAME = "LMS_FUSED_STEP_ANT"
_build_cache = {}


def _make_fused_uops():
    from concourse.dve_uop import (
        UopConfig, UopDpConfig, AluOp, AluInp, InpSel, OutSel, OutPath,
        Trigger, DelayInp, ENABLE,
    )

    PREV = AluInp.PREV_ALU_OUT
    CURR = AluInp.CURR_ALU_OUT
    L0, L1, L2, L3, L4, L5 = (AluInp.PREV_DELAY_0, AluInp.PREV_DELAY_1,
                              AluInp.PREV_DELAY_2, AluInp.PREV_DELAY_3,
                              AluInp.PREV_DELAY_4, AluInp.PREV_DELAY_5)

    def base_uop():
        u = UopConfig()
        u.enable_input(InpSel.SRC_0, 1)     # lane 0
        u.enable_input(InpSel.SRC_1, 2)     # lane 1
        u.enable_input(InpSel.CONST_0, 3)   # lane 2: s_t
        u.enable_input(InpSel.ZERO, 4)      # lane 3: seed zero
        for b in range(8):
            u.datapath_config[b].pass_through_delay(0, 1, 2, 3, 4, 5)
        return u

    def a_blocks(u):
        dp = u.datapath_config
        dp[0].enable_alu(AluOp.MULTIPLY, L0, L1)
        dp[1].enable_alu(AluOp.ADD, CURR, PREV)
        dp[2].enable_alu(AluOp.IS_LT, PREV, L2)       # g1 = pr < s
        dp[2].enable_delay_from_src(DelayInp.PREV_ALU_OUT, 4)   # lane4 <- pr
        dp[3].enable_alu(AluOp.IS_LT, L2, L4)         # g2 = s < pr
        dp[3].enable_delay_from_src(DelayInp.PREV_ALU_OUT, 5)   # lane5 <- g1
        dp[4].enable_alu(AluOp.SUBTRACT, L5, PREV)    # sigma = g1 - g2
        dp[5].enable_alu(AluOp.BYPASS, PREV)
        dp[6].enable_alu(AluOp.BYPASS, PREV)
        dp[7].enable_alu(AluOp.BYPASS, PREV)

    # uop0: seed — zero the pr flop; run the A datapath otherwise; no
    # consume, no write (mirrors lower()'s seed-state conventions).
    seed = base_uop()
    a_blocks(seed)
    seed.datapath_config[1].enable_alu(AluOp.BYPASS, L3, L3)
    seed.trigger = (Trigger.COUNT, Trigger.NONE, Trigger.NONE)
    seed.next_uop = (1, 0, 0)
    seed.repeat_count = 1

    # uop1: A — 3 dot elements, then COUNT -> B
    ua = base_uop()
    a_blocks(ua)
    ua.trigger = (Trigger.SRC_TENSOR_DONE, Trigger.NONE, Trigger.COUNT)
    ua.next_uop = (0, 0, 2)
    ua.repeat_count = 3
    ua.require_inp0 = 1
    ua.require_inp1 = 1
    ua.enable_output(OutSel.ALU_OUT, OutPath.WR0_LO)

    # uop2: B — 3 update elements until the stream ends
    ub = base_uop()
    dp = ub.datapath_config
    dp[0].enable_alu(AluOp.BYPASS, PREV)
    dp[1].enable_alu(AluOp.BYPASS, CURR, CURR)        # freeze pr
    dp[2].enable_alu(AluOp.BYPASS, PREV)
    dp[3].enable_alu(AluOp.BYPASS, PREV)
    dp[4].enable_alu(AluOp.BYPASS, CURR, CURR)        # hold sigma
    dp[5].enable_alu(AluOp.MULTIPLY, PREV, L0)        # sigma * n_p
    dp[6].enable_alu(AluOp.ADD, PREV, L1)             # + w_p
    dp[7].enable_alu(AluOp.BYPASS, PREV)
    ub.trigger = (Trigger.SRC_TENSOR_DONE, Trigger.NONE, Trigger.NONE)
    ub.next_uop = (0, 0, 0)
    ub.require_inp0 = 1
    ub.require_inp1 = 1
    ub.enable_output(OutSel.ALU_OUT, OutPath.WR0_LO)

    uops = [seed, ua, ub]
    for u in uops:
        u.validate("v3")
    return uops


def _register_fused_op():
    """Register the fused op; inject hand-crafted uops into the compile
    cache so table-gen uses them instead of lowering the spec."""
    import concourse.dve_ops as dve_ops
    from concourse.dve_ops import _COMPILE_CACHE
    from concourse.dve_spec import Spec, Src0, Src1, C0, AluOp, scan
    from concourse.dve_uop import DveOpSpec

    for op in dve_ops.OPS:
        if op.name == _OP_NAME:
            return op

    # Spec body documents the A-phase semantic (clipless sigma); it also
    # determines rd1_en (uses Src1) and the scalar slots (C0 only — C2-free
    # so the 2-free-dim in1 STT encoding is allowed). The actual uops are
    # injected below.
    pr = scan(AluOp.ADD, Src0 * Src1)
    body = (C0 > pr) - (C0 < pr)

    def _ref(in0, in1, s0, s1, imm2):
        in0 = np.asarray(in0, np.float32)
        in1 = np.asarray(in1, np.float32)
        pr = np.cumsum((in0 * in1).astype(np.float32), axis=-1,
                       dtype=np.float32)
        s0 = np.asarray(s0, np.float32)
        return (np.greater(s0, pr).astype(np.float32)
                - np.less(s0, pr).astype(np.float32))

    spec = Spec(body=body, reference=_ref)
    row = dve_ops._CUSTOM_DVE_ROW_BASE + len(dve_ops.OPS)
    uops = _make_fused_uops()
    shas = {}
    for ver in ("v3", "v4"):
        compiled = DveOpSpec(name=_OP_NAME, opcode=row, uops=uops, rd1_en=True)
        shas[ver] = compiled.sha(ver)
        _COMPILE_CACHE[(_OP_NAME, ver)] = compiled
    op = dve_ops.DveOp(_OP_NAME, spec, subdim=False, uops_sha=shas)
    dve_ops.OPS.append(op)
    dve_ops.CUSTOM_DVE_SPECS[op.name] = spec
    dve_ops._SUB_OPCODE_FOR_NAME[op.name] = row
    return op


def _restride(ap, free_pairs):
    import bass_rust
    part = ap.ap.to_list()[0]
    ap.ap = bass_rust.VecI64Pair([part] + [list(p) for p in free_pairs])
    return ap


def _strip_dve_self_sems(nc):
    """Drop per-instruction DVE self-semaphore waits (program order + the
    DVE's end-of-op drain already give the ordering); keep all updates."""
    for f in nc.m.functions:
        blocks = list(f.blocks)
        is_dve = lambda i: "DVE" in str(i.engine)
        sem_ids = set()
        for b in blocks:
            for i in b.instructions:
                if is_dve(i) and i.sync_info is not None:
                    for u in i.sync_info.on_update:
                        if u.ant_name and u.ant_name.startswith("DVE"):
                            sem_ids.add(u.id)
        for b in blocks:
            for i in b.instructions:
                si = i.sync_info
                if si is None or not is_dve(i):
                    continue
                if any(w.id in sem_ids for w in si.on_wait):
                    si.on_wait = [w for w in si.on_wait if w.id not in sem_ids]


def _build(T, C, pair_loop, reps=1):
    import concourse.bass as bass
    import concourse.bacc as bacc
    import concourse.mybir as mybir
    from concourse.tile import TileContext

    key = (T, C, pair_loop, reps)
    if key in _build_cache:
        return _build_cache[key]

    op = _register_fused_op()
    nchunks = T // C
    assert T % C == 0 and (nchunks % 2 == 0 or not pair_loop)

    nc = bacc.Bacc(trn_type="TRN2", detect_race_conditions=False)
    f32 = mybir.dt.float32
    add = mybir.AluOpType.add
    mult = mybir.AluOpType.mult
    amin = mybir.AluOpType.min
    amax = mybir.AluOpType.max

    bands = nc.dram_tensor("bands", [ZB + P, T], f32, kind="ExternalInput")
    mubands = nc.dram_tensor("mubands", [ZB + P - 1, T], f32, kind="ExternalInput")
    w0s = nc.dram_tensor("w0s", [ZB, P], f32, kind="ExternalInput")
    predres_o = nc.dram_tensor("predres", [2, ZB, T], f32, kind="ExternalOutput")

    with TileContext(nc) as tc:
        # cmb pages: 0..2 = mu*rows z..z+2, 4..7 = raw rows z..z+3
        #   for band z: mu_n_p = page (2-p), n_p = page (6-p), sample = page 7
        cmb = nc.alloc_sbuf_tensor("cmb", [ZB, 2, 8, C], f32).ap()
        wtraj = nc.alloc_sbuf_tensor("wtraj", [ZB, C + 1, P], f32).ap()
        predres = nc.alloc_sbuf_tensor("predres_sb", [ZB, 2, C], f32).ap()
        prods = nc.alloc_sbuf_tensor("prods", [ZB, C, P], f32).ap()
        a_t = nc.alloc_sbuf_tensor("a", [ZB, C], f32).ap()

        # seed: carry-copy below reads "previous chunk's last w slot"
        nc.sync.dma_start(wtraj[:, C, :], w0s[:, :])

        def do_chunk(col0, par):
            src = _restride(bands[0:ZB, bass.ds(col0, C)].unsqueeze(1),
                            [[T, 4], [1, C]])
            nc.sync.dma_start(cmb[:, par, 4:8, :], src)
            msrc = _restride(mubands[0:ZB, bass.ds(col0, C)].unsqueeze(1),
                             [[T, 3], [1, C]])
            nc.sync.dma_start(cmb[:, par, 0:3, :], msrc)

            s_col = cmb[:, par, 7, :]
            # carry w' into slot 0 (on DVE: keeps the chain single-engine)
            nc.vector.tensor_copy(wtraj[:, 0, :], wtraj[:, C, :])

            # ---- the sequential scan: ONE fused DVE instruction per step ----
            for t in range(C):
                in0 = _restride(cmb[:, par, 2, t:t + 1], [[4 * C, 2], [-C, 3]])
                in1 = _restride(wtraj[:, t, :].unsqueeze(1), [[0, 2], [1, 3]])
                outw = _restride(wtraj[:, t + 1, :].unsqueeze(1),
                                 [[0, 2], [1, 3]])
                nc.vector._custom_dve(
                    op,
                    out=outw,
                    in0=in0,
                    in1=in1,
                    s0=s_col[:, t:t + 1],
                    s1=0.0,
                )

            # ---- bulk reconstruction off the DVE (Pool engine) ----
            mbr_cp = _restride(cmb[:, par, 2, 0:C], [[1, C], [-C, 3]])
            nc.gpsimd.tensor_tensor(prods[:, :, :], mbr_cp,
                                    wtraj[:, 0:C, :], mult)
            nc.gpsimd.tensor_tensor(a_t[:, :], prods[:, :, 0], prods[:, :, 1], add)
            nc.gpsimd.tensor_tensor(a_t[:, :], a_t[:, :], prods[:, :, 2], add)
            nc.gpsimd.tensor_scalar(predres[:, 0, :], a_t[:, :],
                                    32767.0, -32768.0, op0=amin, op1=amax)
            nc.gpsimd.tensor_tensor(predres[:, 1, :], s_col,
                                    predres[:, 0, :], mybir.AluOpType.subtract)
            dst = _restride(predres_o[0, 0:ZB, bass.ds(col0, C)].unsqueeze(1),
                            [[ZB * T, 2], [1, C]])
            nc.sync.dma_start(dst, predres[:, :, :])

        def do_pair(base):
            do_chunk(base, 0)
            do_chunk(base + C, 1)

        if pair_loop and reps > 1:
            with tc.For_i(0, reps, 1):
                with tc.For_i(0, nchunks // 2, 1) as i:
                    do_pair(i * (2 * C))
        elif pair_loop:
            with tc.For_i(0, nchunks // 2, 1) as i:
                do_pair(i * (2 * C))
        else:
            for k in range(nchunks // 2):
                do_pair(k * (2 * C))

    _strip_dve_self_sems(nc)
    if not nc.is_finalized():
        nc.finalize()
    _build_cache[key] = nc
    return nc


def _run_spmd(nc, in_maps, trace=False):
    from concourse.bass_utils import run_bass_kernel_spmd
    return run_bass_kernel_spmd(nc, in_maps, core_ids=list(range(len(in_maps))),
                                trace=trace)


def _make_in_maps(G, w0p):
    mug = (np.float32(MU) * G).astype(np.float32)
    in_maps = []
    for c in range(NCORES):
        in_maps.append({
            "bands": np.ascontiguousarray(G[ZB * c:ZB * c + ZB + P]),
            "mubands": np.ascontiguousarray(mug[ZB * c:ZB * c + ZB + P - 1]),
            "w0s": np.ascontiguousarray(w0p[ZB * c:ZB * c + ZB]),
        })
    return in_maps


def kernel(image, w0):
    image = np.ascontiguousarray(np.asarray(image, np.float32))
    w0 = np.asarray(w0, np.float32)
    Z = image.shape[0]
    img_flat = image.reshape(Z, T_FULL)
    assert float(np.abs(img_flat).max()) < 32000.0, \
        "clipless sigma requires |s| < 32768"

    G = np.zeros((P + ZPAD, T_FULL), np.float32)
    G[P:P + Z] = img_flat
    w0p = np.zeros((ZPAD, P), np.float32)
    w0p[:Z] = w0 * np.float32(1024.0)

    nc = _build(T_FULL, 2048, True)
    results = _run_spmd(nc, _make_in_maps(G, w0p)).results

    preds = np.concatenate([r["predres"][0] for r in results], axis=0)[:Z]
    ress = np.concatenate([r["predres"][1] for r in results], axis=0)[:Z]
    predictions = preds.reshape(Z, Y, X)
    residuals = ress.reshape(Z, Y, X)

    q = np.rint(residuals).astype(np.int32)
    mapped = np.where(q >= 0, 2 * q, -2 * q - 1).astype(np.int32)
    reconstructed = predictions + residuals
    return (predictions, residuals, residuals, mapped, image, reconstructed)
